# revision 41
# baseline (speedup 1.0000x reference)
"""Trainium2 Bass kernel for DifferentiableLengthRegulator.

Math (per batch b):
  center = cumsum(w) - 0.5*w                          [T]
  delta  = clip(pos - center[:,None], 1e-4, 1e4)      [T, L]
  W      = exp(-0.5 * (delta*w)^2 * sigma_scale)      [T, L]   (in (0, 1])
  P      = softmax_T(masked(W))                       [T, L]
  out    = (x * x_mask) @ P * y_mask                  [C, L]

Since softmax is over already-exponentiated W in [0,1], no max-subtraction is
needed: P = V / sum_T V with V = exp(W) in [1, e].  We compute
  num = (x*x_mask) @ V,  den = x_mask @ V,  out = num * (1/den) * y_mask.

Sharding: data-parallel over batch, 4 batches per core, 8 cores, no collectives.

Device-side structure per (batch, T-tile of 128 rows):
  V row-tile [128, L] is piecewise:
    l <  lo: pos-center < 1e-4 everywhere -> V = vA_row (row constant)  (cheap fill)
    l >= hi: W < 1e-8 -> V = 1                                          (memset)
    else   : dense: t = max(pos-center, 1e-4); V = exp(exp(-c*t^2))     (DVE+ACT)
  [lo, hi) bounds are a union over all 32 batches (single SPMD program).
  T rows are permuted per batch on the host (wide-band rows parked last) to
  tighten the bounds; permuting T is free since both softmax and the matmul
  contract over T.

Host does only O(B*T) prep (cumsum, masks, permutation) plus shard/gather.
"""

import numpy as np

_B, _C, _T, _L = 32, 256, 512, 2048
_NC = 8
_BPC = _B // _NC          # batches per core
_TI = _T // 128           # T tiles per batch
_CH = 512                 # matmul N-chunk (one PSUM bank, fp32)
_LJ = _L // _CH
_WIDE_CUT = 256.0         # rows with band wider than this go to the back
_W_THRESH = 18.42
_PURE_SKIP = False         # exp(-18.42) ~ 1e-8: treat W as 0 beyond this

LAST_RESULT = None        # BassKernelResults of the last run (for test harness)


_CLAMP_SQ = None


def _get_clamp_sq():
    """out = square(max(in0 - s0, s1)) as one custom DVE op."""
    global _CLAMP_SQ
    if _CLAMP_SQ is not None:
        return _CLAMP_SQ
    import numpy as np
    import concourse.dve_ops as dops
    from concourse.dve_spec import Spec, Src0, C0, C1, sq, maxx, lower
    from concourse.dve_ops import has_src1, DveOpSpec

    spec = Spec(
        body=sq(maxx(Src0 - C0, C1)),
        reference=lambda in0, in1, s0, s1, imm2: np.square(
            np.maximum(in0 - s0, s1)),
    )
    op = dops.DveOp("CLAMP_SQ_ANT", spec, subdim=False, uops_sha={})
    row = max(dops._SUB_OPCODE_FOR_NAME.values()) + 1
    assert row < 0x20
    dops.OPS.append(op)
    dops.CUSTOM_DVE_SPECS[op.name] = spec
    dops._SUB_OPCODE_FOR_NAME[op.name] = row
    for ver in ("v3", "v4"):
        s2 = DveOpSpec(name=op.name, opcode=row,
                            uops=lower(spec, ver=ver),
                            rd1_en=has_src1(spec))
        op.uops_sha[ver] = s2.sha(ver)
    _CLAMP_SQ = op
    return op


def _install_trace_shim():
    """Make run_bass_kernel_spmd(trace=True) work in the agent container,
    where antenv.axon_hooks is not injected."""
    import sys
    import types

    try:
        from antenv.axon_hooks import get_axon_ntff_profile_hook  # noqa: F401
        return
    except ImportError:
        pass
    from trn_agent_boot.trn_boot import _ntff_profile_via_ctypes

    hook = _ntff_profile_via_ctypes("/opt/axon/libaxon_pjrt.so")
    mod = types.ModuleType("antenv.axon_hooks")
    mod.get_axon_ntff_profile_hook = lambda: hook
    mod.set_axon_ntff_profile_hook = lambda h: None
    sys.modules["antenv.axon_hooks"] = mod

    import concourse.bass_utils as bu

    bu.upload_artifacts = lambda tmpdir: f"local://{tmpdir}"


def _build_and_run(xp, ccol, wcol, mcol, acol, ymk, bounds, pure, nb, db,
                   ym_trivial, trace=False, tmpdir=None):
    from contextlib import ExitStack

    import concourse.bass as bass
    import concourse.tile as tile
    from concourse import bacc, mybir
    from concourse.bass_utils import run_bass_kernel_spmd
    from concourse.masks import make_identity

    f32 = mybir.dt.float32
    f32r = mybir.dt.float32r
    f16 = mybir.dt.float16
    Alu = mybir.AluOpType
    Act = mybir.ActivationFunctionType

    clamp_sq = _get_clamp_sq()
    nc = bacc.Bacc("TRN2", target_bir_lowering=False, debug=False,
                   num_devices=_NC)
    xin = nc.dram_tensor("xin", [_BPC, _C, _T], f16, kind="ExternalInput")
    ccol_d = nc.dram_tensor("ccol", [128, _BPC * _TI], f32, kind="ExternalInput")
    wcol_d = nc.dram_tensor("wcol", [128, _BPC * _TI], f32, kind="ExternalInput")
    mcol_d = nc.dram_tensor("mcol", [128, _BPC * _TI], f32, kind="ExternalInput")
    acol_d = nc.dram_tensor("acol", [128, _BPC * _TI], f32, kind="ExternalInput")
    ym_d = nc.dram_tensor("ymk", [1, _BPC * _L], f32, kind="ExternalInput")
    nb_d = nc.dram_tensor("nb", [1, _BPC * _LJ * _C], f32, kind="ExternalInput")
    db_d = nc.dram_tensor("db", [1, _BPC * _LJ], f32, kind="ExternalInput")
    out_d = nc.dram_tensor("out", [_BPC, _C, _L], f16, kind="ExternalOutput")

    with tile.TileContext(nc) as tc, ExitStack() as ctx:
        singles = ctx.enter_context(tc.tile_pool(name="singles", bufs=1))
        xn_pool = ctx.enter_context(tc.tile_pool(name="xn", bufs=4))
        xt_pool = ctx.enter_context(tc.tile_pool(name="xt", bufs=2 * _TI))
        vv_pool = ctx.enter_context(tc.tile_pool(name="vv", bufs=3 * _TI))
        sc_pool = ctx.enter_context(tc.tile_pool(name="scp", bufs=3))
        rr_pool = ctx.enter_context(tc.tile_pool(name="rr", bufs=2))
        rb_pool = ctx.enter_context(tc.tile_pool(name="rb", bufs=3))
        ob_pool = ctx.enter_context(tc.tile_pool(name="ob", bufs=4))
        dr_pool = ctx.enter_context(tc.tile_pool(name="dr", bufs=4, space="DRAM"))
        pnum = ctx.enter_context(tc.tile_pool(name="pnum", bufs=4, space="PSUM"))
        pden = ctx.enter_context(tc.tile_pool(name="pden", bufs=4, space="PSUM"))

        ident = singles.tile([128, 128], f16)
        make_identity(nc, ident[:])
        iota_f = singles.tile([128, _L], f32)
        nc.gpsimd.iota(iota_f[:], pattern=[[1, _L]], base=0,
                       channel_multiplier=0,
                       allow_small_or_imprecise_dtypes=True)

        ccol_t = singles.tile([128, _BPC * _TI], f32)
        nc.sync.dma_start(out=ccol_t[:], in_=ccol_d[:])
        wcol_t = singles.tile([128, _BPC * _TI], f32)
        nc.sync.dma_start(out=wcol_t[:], in_=wcol_d[:])
        mcol_t = singles.tile([128, _BPC * _TI], f32)
        nc.sync.dma_start(out=mcol_t[:], in_=mcol_d[:])
        acol_t = singles.tile([128, _BPC * _TI], f32)
        nc.sync.dma_start(out=acol_t[:], in_=acol_d[:])
        if not ym_trivial:
            ym_t = singles.tile([1, _BPC * _L], f32)
            nc.sync.dma_start(out=ym_t[:], in_=ym_d[:])
        mden_t = singles.tile([128, _BPC * _TI], f16)
        nc.scalar.copy(out=mden_t[:], in_=mcol_t[:])
        nb_raw = singles.tile([1, _BPC * _LJ * _C], f32)
        nc.sync.dma_start(out=nb_raw[:], in_=nb_d[:])
        nb_t = singles.tile([1, _BPC * _LJ * _C], f16)
        nc.scalar.copy(out=nb_t[:], in_=nb_raw[:])
        db_raw = singles.tile([1, _BPC * _LJ], f32)
        nc.sync.dma_start(out=db_raw[:], in_=db_d[:])
        db_t = singles.tile([1, _BPC * _LJ], f16)
        nc.scalar.copy(out=db_t[:], in_=db_raw[:])
        onesr_t = singles.tile([1, _CH], f16)
        nc.vector.memset(onesr_t[:], 1.0)

        _load = {"dve": 0.0, "gps": 0.0}

        def prep(bb):
            """load + transpose x, build V tiles, den matmuls, 1/den chain"""
            xnat = []
            for ci in range(2):
                t = xn_pool.tile([128, _T], f16, tag="xnat", name="xnat")
                nc.sync.dma_start(out=t[:], in_=xin[bb, ci * 128:(ci + 1) * 128, :])
                xnat.append(t)
            xT = []
            for ti in range(_TI):
                xt_t = xt_pool.tile([128, _C], f16, tag="xT", name="xT")
                for ci in range(2):
                    pt = pnum.tile([128, _CH], f16, tag="pp", name="pt")
                    nc.tensor.transpose(
                        pt[:, 0:128], xnat[ci][:, ti * 128:(ti + 1) * 128],
                        ident[:])
                    nc.scalar.copy(out=xt_t[:, ci * 128:(ci + 1) * 128],
                                   in_=pt[:, 0:128])
                xT.append(xt_t)

            vts = [None] * _TI
            order = sorted(range(_TI),
                           key=lambda t: (t != 0,
                                          bounds[bb][t][0] - bounds[bb][t][1]))
            for ti in order:
                bt = bb * _TI + ti
                lo, hi = bounds[bb][ti]
                vt = vv_pool.tile([128, _L], f16, tag="vt", name="vt")
                if lo > 0:
                    # left region: V = vA row constant  (iota*0 + vA)
                    eng = nc.gpsimd
                    eng.tensor_scalar(
                        out=vt[:, 0:lo], in0=iota_f[:, 0:lo],
                        scalar1=0.0, scalar2=acol_t[:, bt:bt + 1],
                        op0=Alu.mult, op1=Alu.add)
                if hi < _L:
                    eng = nc.gpsimd
                    eng.tensor_scalar(
                        out=vt[:, hi:_L], in0=iota_f[:, hi:_L],
                        scalar1=0.0, scalar2=1.0,
                        op0=Alu.mult, op1=Alu.add)
                if hi > lo:
                    sc = sc_pool.tile([128, max(hi - lo, 8)], f32,
                                      tag=f"sc{ti}", name="sc")
                    # split long bands into independent chains for overlap
                    nsub = max(1, min(3, (hi - lo) // 700))
                    step = ((hi - lo) // nsub + 7) // 8 * 8
                    subs = []
                    a = lo
                    while a < hi:
                        b = min(a + step, hi)
                        subs.append((a, b))
                        a = b
                    for (a, b) in subs:
                        sa, sb = a - lo, b - lo
                        # t2 = max(pos - center, 1e-4)^2: fused DVE op, or the
                        # 2-op path on the (otherwise idle) gpsimd engine
                        if True:
                            nc.vector._custom_dve(
                                clamp_sq, out=sc[:, sa:sb],
                                in0=iota_f[:, a:b],
                                s0=ccol_t[:, bt:bt + 1], s1=1e-4)
                            _load["dve"] += (b - a)
                        else:
                            nc.gpsimd.tensor_scalar(
                                out=sc[:, sa:sb], in0=iota_f[:, a:b],
                                scalar1=ccol_t[:, bt:bt + 1], scalar2=1e-4,
                                op0=Alu.subtract, op1=Alu.max)
                            nc.gpsimd.tensor_tensor(
                                out=sc[:, sa:sb], in0=sc[:, sa:sb],
                                in1=sc[:, sa:sb], op=Alu.mult)
                            _load["gps"] += (b - a)
                        # W = exp(-0.5*s*w^2 * t2)   (per-row scale)
                        nc.scalar.activation(
                            out=sc[:, sa:sb], in_=sc[:, sa:sb], func=Act.Exp,
                            scale=wcol_t[:, bt:bt + 1])
                        # V = exp(W)
                        nc.scalar.activation(
                            out=vt[:, a:b], in_=sc[:, sa:sb], func=Act.Exp)
                vts[ti] = vt

            # den matmuls (their 1/den tails overlap the num phase)
            band = {lj: [ti for ti in range(_TI) if pure[bb][ti][lj] is None]
                    for lj in range(_LJ)}
            pds = []
            for lj in range(_LJ):
                pds.append(pden.tile([1, _CH], f32, tag="pden", name=f"pd{lj}"))
                if len(band[lj]) < _TI:
                    # init with the host-computed pure-cell contribution
                    k = bb * _LJ + lj
                    nc.tensor.matmul(
                        pds[lj][:], db_t[0:1, k:k + 1],
                        onesr_t[:], start=True,
                        stop=not band[lj])
            for ti in range(_TI):
                bt = bb * _TI + ti
                for lj in range(_LJ):
                    if pure[bb][ti][lj] is not None:
                        continue
                    sl = slice(lj * _CH, (lj + 1) * _CH)
                    nc.tensor.matmul(
                        pds[lj][:], mden_t[:, bt:bt + 1],
                        vts[ti][:, sl],
                        start=(len(band[lj]) == _TI and ti == band[lj][0]),
                        stop=(ti == band[lj][-1]))
            r_row = rr_pool.tile([1, _L], f32, tag="r", name="r")
            for lj in range(_LJ):
                sl = slice(lj * _CH, (lj + 1) * _CH)
                # r = (1/den) * y_mask
                nc.vector.reciprocal_approx_fast(out=r_row[0:1, sl],
                                                 in_=pds[lj][:])
                if not ym_trivial:
                    nc.gpsimd.tensor_tensor(
                        out=r_row[0:1, sl], in0=r_row[0:1, sl],
                        in1=ym_t[0:1, bb * _L + lj * _CH:bb * _L + (lj + 1) * _CH],
                        op=Alu.mult)
            # broadcast over 128 partitions: bounce via DRAM, then a
            # stride-0-partition DMA read (legal for DRAM sources)
            r2d = dr_pool.tile([1, _L], f32, tag="r2d", name="r2d")
            nc.sync.dma_start(out=r2d[:], in_=r_row[:])
            rbt = rb_pool.tile([128, _L], f32, tag="rb", name="rbt")
            r2b = bass.AP(tensor=r2d.tensor, offset=r2d.offset,
                          ap=[[0, 128], r2d.ap[-1]])
            nc.sync.dma_start(out=rbt[:], in_=r2b)
            return xT, vts, rbt

        def main(bb, st):
            xT, vts, rbt = st
            band = {lj: [ti for ti in range(_TI) if pure[bb][ti][lj] is None]
                    for lj in range(_LJ)}
            ob = []
            for ci in range(2):
                ob.append(ob_pool.tile([128, _L], f16, tag="ob", name=f"ob{ci}"))
            for ljp in range(_LJ // 2):
                ljs = (2 * ljp, 2 * ljp + 1)
                pn = {(ci, lj): pnum.tile([128, _CH], f32, tag="pp",
                                          name=f"pn{ci}{lj}")
                      for ci in range(2) for lj in ljs}
                for ci in range(2):
                    for lj in ljs:
                        if len(band[lj]) < _TI:
                            k = (bb * _LJ + lj) * _C + ci * 128
                            nc.tensor.matmul(
                                pn[ci, lj][:],
                                nb_t[0:1, k:k + 128],
                                onesr_t[:], start=True,
                                stop=not band[lj])
                for ti in range(_TI):
                    for ci in range(2):
                        for lj in ljs:
                            if pure[bb][ti][lj] is not None:
                                continue
                            sl = slice(lj * _CH, (lj + 1) * _CH)
                            nc.tensor.matmul(
                                pn[ci, lj][:],
                                xT[ti][:, ci * 128:(ci + 1) * 128],
                                vts[ti][:, sl],
                                start=(len(band[lj]) == _TI
                                       and ti == band[lj][0]),
                                stop=(ti == band[lj][-1]))
                for lj in ljs:
                    sl = slice(lj * _CH, (lj + 1) * _CH)
                    for ci in range(2):
                        nc.vector.tensor_tensor(out=ob[ci][:, sl],
                                                in0=pn[ci, lj][:],
                                                in1=rbt[:, sl], op=Alu.mult)
                slp = slice(ljs[0] * _CH, (ljs[1] + 1) * _CH)
                for ci in range(2):
                    nc.sync.dma_start(
                        out=out_d[bb, ci * 128:(ci + 1) * 128, slp],
                        in_=ob[ci][:, slp])

        # 1-batch software pipeline: prep(b+1) is emitted before main(b) so
        # each engine's static instruction stream interleaves batches
        blen = [sum(h - l for (l, h) in bounds[bb]) for bb in range(_BPC)]
        sched = sorted(range(_BPC), key=lambda bb: -blen[bb])
        states = {}
        states[sched[0]] = prep(sched[0])
        for k, bb in enumerate(sched):
            if k + 1 < _BPC:
                states[sched[k + 1]] = prep(sched[k + 1])
            main(bb, states.pop(bb))

    nc.compile()

    in_maps = []
    for i in range(_NC):
        in_maps.append({
            "xin": xp[i], "ccol": ccol[i], "wcol": wcol[i],
            "mcol": mcol[i], "acol": acol[i], "ymk": ymk[i],
            "nb": nb[i], "db": db[i],
        })
    kwargs = {}
    if trace:
        _install_trace_shim()
        if tmpdir is not None:
            kwargs["tmpdir"] = tmpdir
    return run_bass_kernel_spmd(nc, in_maps, list(range(_NC)), trace=trace,
                                **kwargs)


def kernel(x, w, x_mask, y_mask, sigma_scale, _trace=False, _tmpdir=None):
    global LAST_RESULT
    x = np.ascontiguousarray(np.asarray(x, dtype=np.float32))
    w_ = np.asarray(w, dtype=np.float32)
    xm = np.asarray(x_mask, dtype=np.float32).reshape(_B, _T)
    ym = np.asarray(y_mask, dtype=np.float32).reshape(_B, _L)
    s = float(np.asarray(sigma_scale, dtype=np.float64).reshape(-1)[0])

    # host prep: O(B*T)
    center = np.cumsum(w_, axis=1, dtype=np.float32) - np.float32(0.5) * w_
    wsc2 = 0.5 * s * w_.astype(np.float64) ** 2          # W = exp(-wsc2 * t^2)
    with np.errstate(divide="ignore"):
        cut = np.where(wsc2 > 0, np.sqrt(_W_THRESH / np.maximum(wsc2, 1e-300)),
                       np.inf)
    vA = np.exp(np.exp(-wsc2 * 1e-8)).astype(np.float32)  # V at delta=1e-4

    # per-batch T permutation: wide-band rows last, otherwise natural order
    perm = np.empty((_B, _T), np.int64)
    for b in range(_B):
        wide = cut[b] > _WIDE_CUT
        perm[b] = np.argsort(wide, kind="stable")

    center_p = np.take_along_axis(center, perm, axis=1)
    cut_p = np.take_along_axis(cut, perm, axis=1)
    wsc2_p = np.take_along_axis(wsc2, perm, axis=1)
    xm_p = np.take_along_axis(xm, perm, axis=1)
    vA_p = np.take_along_axis(vA, perm, axis=1)

    # assign batches to (core, slot) so that the 8 batches sharing a slot have
    # similar center curves -> tighter union bounds per slot
    order = np.argsort(center[:, _T // 2], kind="stable")
    # slot bb gets batches order[bb*8:(bb+1)*8]; core i gets the i-th of each
    assign = np.empty((_NC, _BPC), np.int64)  # assign[i, bb] = source batch
    for bb in range(_BPC):
        for i in range(_NC):
            assign[i, bb] = order[bb * _NC + i]

    # union [lo, hi) bounds per (slot, T-tile) over the slot's 8 batches
    bounds = []
    for bb in range(_BPC):
        grp = [int(assign[i, bb]) for i in range(_NC)]
        row = []
        for ti in range(_TI):
            slt = slice(ti * 128, (ti + 1) * 128)
            cmin = float(center_p[grp][:, slt].min())
            cmax = float(np.minimum(center_p[grp][:, slt] + cut_p[grp][:, slt],
                                    1e18).max())
            lo = int(np.clip((np.floor(cmin + 1e-4 - 1e-3) // 8) * 8, 0, _L))
            hi = int(np.clip(np.ceil((cmax + 1e-3) / 8) * 8, lo, _L))
            row.append((lo, hi))
        bounds.append(row)

    # pure-cell classification per (slot, ti, lj): chunk entirely left of lo
    # (A: V = vA row const) or right of hi (C: V = 1) for ALL slot batches
    pure = [[[None] * _LJ for _ in range(_TI)] for _ in range(_BPC)]
    if _PURE_SKIP:
        for bb in range(_BPC):
            for ti in range(_TI):
                lo, hi = bounds[bb][ti]
                for lj in range(_LJ):
                    if (lj + 1) * _CH <= lo:
                        pure[bb][ti][lj] = "A"
                    elif lj * _CH >= hi:
                        pure[bb][ti][lj] = "C"

    # base contributions of pure cells, computed on host in float64
    # nbase[i, bb, lj, ci*128+c] = sum over pure tiles' rows of x*vA or x*1
    nbase = np.zeros((_NC, _BPC, _LJ, _C), np.float64)
    dbase = np.zeros((_NC, _BPC, _LJ), np.float64)

    # per-core arrays
    xp = np.empty((_NC, _BPC, _C, _T), np.float16)
    ccol = np.empty((_NC, 128, _BPC * _TI), np.float32)
    wcol = np.empty((_NC, 128, _BPC * _TI), np.float32)
    mcol = np.empty((_NC, 128, _BPC * _TI), np.float32)
    acol = np.empty((_NC, 128, _BPC * _TI), np.float32)
    ymk = np.empty((_NC, 1, _BPC * _L), np.float32)
    for i in range(_NC):
        for bb in range(_BPC):
            b = int(assign[i, bb])
            xp[i, bb] = (x[b] * xm[b][None, :])[:, perm[b]]
            ymk[i, 0, bb * _L:(bb + 1) * _L] = ym[b]
            xb = (x[b] * xm[b][None, :])[:, perm[b]].astype(np.float64)
            for ti in range(_TI):
                bt = bb * _TI + ti
                slt = slice(ti * 128, (ti + 1) * 128)
                ccol[i, :, bt] = center_p[b, slt]
                wcol[i, :, bt] = (-wsc2_p[b, slt]).astype(np.float32)
                mcol[i, :, bt] = xm_p[b, slt]
                acol[i, :, bt] = vA_p[b, slt]
                va_rows = (vA_p[b, slt] * xm_p[b, slt]).astype(np.float64)
                m_rows = xm_p[b, slt].astype(np.float64)
                xm_rows = xb[:, slt] * m_rows[None, :]
                for lj in range(_LJ):
                    p = pure[bb][ti][lj]
                    if p == "A":
                        nbase[i, bb, lj] += xm_rows @ va_rows
                        dbase[i, bb, lj] += float((m_rows * va_rows).sum())
                    elif p == "C":
                        nbase[i, bb, lj] += xm_rows.sum(axis=1)
                        dbase[i, bb, lj] += float(m_rows.sum())

    nb = nbase.astype(np.float32).reshape(_NC, 1, _BPC * _LJ * _C)
    db = dbase.astype(np.float32).reshape(_NC, 1, _BPC * _LJ)
    res = _build_and_run(xp, ccol, wcol, mcol, acol, ymk, bounds, pure,
                         nb, db, bool(np.all(ym == 1.0)),
                         trace=_trace, tmpdir=_tmpdir)
    LAST_RESULT = res

    out = np.empty((_B, _C, _L), np.float32)
    for i in range(_NC):
        for bb in range(_BPC):
            out[int(assign[i, bb])] = res.results[i]["out"][bb]
    return out
